# revision 9
# baseline (speedup 1.0000x reference)
"""Grouped GEMM (MoE expert matmul) on 8 TRN2 NeuronCores.

Problem: a [66048, 1024] f32 tokens, b [8, 1024, 1024] f32 expert weights,
static uneven per-expert token counts. d[m] = a[m] @ b[expert(m)].

Strategy (expert-parallel via M-sharding, zero collectives):
- Token rows are assigned host-side to 8 cores x 3 "slots" of (6, 22, 37)
  m-tiles (128 rows each) = 65 tiles/core. Every slot is single-expert;
  each core receives the 3 expert matrices its slots need. The
  (core,slot)->expert binding is pure DATA, so one SPMD program serves
  all cores. Only 4 of 520 tiles are zero-padding.
- A is pre-transposed host-side into per-tile lhsT layout [ki, ko, mm]
  (so the PE does no transposes at all) and split into fp8-e4m3 hi+lo
  (a ~= a_h + a_l); B likewise. The product is computed as
      d ~= a_h@b_h + a_l@b_h + a_h@b_l
  with all three terms as fp8 DoubleRow matmuls (2 k-tiles per
  instruction) accumulating into the same PSUM bank. This is 0.75x the
  PE cost of a bf16 kernel at bf16-level accuracy (~2e-3 rel err).
- Per m-tile: 24 DoubleRow matmuls (2 psum halves x 4 k-pairs x 3
  terms), PSUM evicted to SBUF as bf16 by ScalarE/DVE (one half each),
  stored by HWDGE DMA. A tiles stream in chunks of 5 via SWDGE with
  3-chunk prefetch; d is upcast to f32 host-side.
"""

import numpy as np

GROUP_SIZES = [12288, 10240, 9216, 8192, 7168, 7168, 6144, 5632]
OFFSETS = np.concatenate([[0], np.cumsum(GROUP_SIZES)]).astype(np.int64)
M_TOTAL = int(OFFSETS[-1])  # 66048
K = 1024
N = 1024
E = 8
P = 128
KK = K // P  # 8 k-tiles
NH = 2  # two 512-wide psum halves

# Per-core uniform slot structure, in m-tiles of 128 rows.
SLOT_TILES = (6, 22, 37)  # sum = 65 tiles = 8320 rows per core
TILES_PER_CORE = sum(SLOT_TILES)
ROWS_PER_CORE = TILES_PER_CORE * P
SLOT_ROW_OFF = (0, SLOT_TILES[0] * P, (SLOT_TILES[0] + SLOT_TILES[1]) * P)

CHUNK = 5  # m-tiles per A-load DMA; 13 chunks cover 65 tiles
NCHUNKS = TILES_PER_CORE // CHUNK
PREFETCH = 3  # chunks issued ahead of consumption
WARMUP = 110  # dummy PE matmuls burning the p-state ramp during startup DMA

# expert id for (slot, core): found by exact-cover search; 4 pad tiles total.
SLOT_EXPERT = (
    (1, 3, 4, 4, 5, 5, 6, 6),  # slot 0: 6 tiles each
    (0, 3, 4, 4, 5, 5, 7, 7),  # slot 1: 22 tiles each
    (0, 0, 1, 1, 2, 2, 3, 6),  # slot 2: 37 tiles each
)


def _build_schedule():
    """Returns list of (core, slot, slot_row_start, global_row_start, nrows)."""
    cursor = [int(OFFSETS[e]) for e in range(E)]
    recs = []
    # Deterministic fill order: slot index, then core.
    for s in range(3):
        for c in range(8):
            e = SLOT_EXPERT[s][c]
            cap = SLOT_TILES[s] * P
            take = min(cap, int(OFFSETS[e + 1]) - cursor[e])
            if take > 0:
                recs.append((c, s, SLOT_ROW_OFF[s], cursor[e], take))
                cursor[e] += take
    for e in range(E):
        assert cursor[e] == int(OFFSETS[e + 1]), (e, cursor[e])
    return recs


_SCHEDULE = _build_schedule()


def _build_bass():
    import concourse.bass as bass  # noqa: F401
    import concourse.mybir as mybir
    import concourse.tile as tile
    from concourse import bacc

    f32 = mybir.dt.float32
    bf16 = mybir.dt.bfloat16
    f8 = mybir.dt.float8e4

    nc = bacc.Bacc(
        "TRN2", target_bir_lowering=False, debug=False, enable_asserts=False
    )

    # A in pre-transposed lhsT layout: row (t*128 + ki) holds the 1024
    # values [ko, mm] of tile t; hi and lo fp8 planes.
    ah = nc.dram_tensor("ah", [ROWS_PER_CORE, K], f8, kind="ExternalInput").ap()
    al = nc.dram_tensor("al", [ROWS_PER_CORE, K], f8, kind="ExternalInput").ap()
    # B per slot: row (ki*8 + ko) holds the 1024 n-values; hi and lo.
    bhs = [
        nc.dram_tensor(f"bh{s}", [P * KK, N], f8, kind="ExternalInput").ap()
        for s in range(3)
    ]
    bls = [
        nc.dram_tensor(f"bl{s}", [P * KK, N], f8, kind="ExternalInput").ap()
        for s in range(3)
    ]
    d = nc.dram_tensor("d", [ROWS_PER_CORE, N], bf16, kind="ExternalOutput").ap()

    # which slot (-> b input) each m-tile uses (static, uniform across cores)
    tile_slot = []
    for s in range(3):
        tile_slot += [s] * SLOT_TILES[s]

    from contextlib import ExitStack

    with tile.TileContext(nc) as tc, ExitStack() as ctx:
        bpool = ctx.enter_context(tc.tile_pool(name="bpool", bufs=1))
        ahpool = ctx.enter_context(tc.tile_pool(name="ahpool", bufs=4))
        alpool = ctx.enter_context(tc.tile_pool(name="alpool", bufs=4))
        psd = ctx.enter_context(tc.tile_pool(name="psd", bufs=7, space="PSUM"))
        wps = ctx.enter_context(tc.tile_pool(name="wps", bufs=1, space="PSUM"))
        # Deep store staging: early DMA-engine time is monopolized by the
        # B/A loads, so d-stores queue up; 24 bufs (48KB) of slack keep the
        # eviction copies (and thus PSUM recycling) from backpressuring PE.
        dpool = ctx.enter_context(tc.tile_pool(name="dpool", bufs=24))

        # Warmup: the PE p-state ramps to full clock only after 3us of
        # continuous execution. Dummy DoubleRow matmuls on zeroed tiles
        # keep the PE busy (and ramping) while the first B/A DMAs land,
        # so the real matmul stream starts at full speed with no idle gap.
        warm = ctx.enter_context(tc.tile_pool(name="warm", bufs=1))
        wa = warm.tile([P, 2, P], f8, name="wa")
        wb = warm.tile([P, 2, 256], f8, name="wb")
        nc.vector.memset(wa[:], 0.0)
        nc.vector.memset(wb[:], 0.0)
        wp = wps.tile([P, 256], f32, name="wp")
        for _ in range(WARMUP):
            nc.tensor.matmul(
                wp[:],
                wa[:],
                wb[:],
                start=True,
                stop=True,
                perf_mode=mybir.MatmulPerfMode.DoubleRow,
            )

        b_sb = {}  # (slot, lvl) -> [128, KK, N] fp8 tile

        def load_b(s, lvl):
            src = (bhs if lvl == 0 else bls)[s]
            bt = bpool.tile([P, KK, N], f8, name=f"b{lvl}_{s}")
            nc.gpsimd.dma_start(
                out=bt[:], in_=src.rearrange("(ki ko) n -> ki ko n", ko=KK)
            )
            b_sb[(s, lvl)] = bt

        a_ch = {}  # (chunk, lvl) -> [128, CHUNK, KK, 128] fp8 tile

        def load_chunk(c):
            for lvl, (pool, src) in enumerate(((ahpool, ah), (alpool, al))):
                at = pool.tile([P, CHUNK, KK, P], f8, name=f"a{lvl}")
                nc.gpsimd.dma_start(
                    out=at[:],
                    in_=src[c * CHUNK * P : (c + 1) * CHUNK * P, :].rearrange(
                        "(c ki) (ko mm) -> ki c ko mm", ki=P, ko=KK
                    ),
                )
                a_ch[(c, lvl)] = at

        # Startup: b for slot 0 first, then the first A chunks. The other
        # b loads are deferred into the loop (slot 1 is needed at tile 6,
        # slot 2 at tile 28) so the early DMA engine time is free for the
        # first d-stores instead of being monopolized by weight loads.
        load_b(0, 0)
        load_chunk(0)
        load_b(0, 1)
        load_chunk(1)
        load_chunk(2)

        for t in range(TILES_PER_CORE):
            c, j = divmod(t, CHUNK)
            if j == 0 and c + PREFETCH < NCHUNKS:
                load_chunk(c + PREFETCH)
            if t == 1:
                load_b(1, 0)
                load_b(1, 1)
            if t == 8:
                load_b(2, 0)
                load_b(2, 1)
            s = tile_slot[t]
            at_h = a_ch[(c, 0)]
            at_l = a_ch[(c, 1)]
            b_h = b_sb[(s, 0)]
            b_l = b_sb[(s, 1)]
            ps = [psd.tile([P, 512], f32, name="ps") for _ in range(NH)]
            for nh in range(NH):
                n0, n1 = nh * 512, (nh + 1) * 512
                idx = 0
                for w_t, r_t in ((at_h, b_h), (at_l, b_h), (at_h, b_l)):
                    for jj in range(KK // 2):
                        nc.tensor.matmul(
                            ps[nh][:],
                            w_t[:, j, 2 * jj : 2 * jj + 2, :],
                            r_t[:, 2 * jj : 2 * jj + 2, n0:n1],
                            start=(idx == 0),
                            stop=(idx == 3 * KK // 2 - 1),
                            perf_mode=mybir.MatmulPerfMode.DoubleRow,
                        )
                        idx += 1
            d_sb = dpool.tile([P, N], bf16, name="d_sb")
            if t < TILES_PER_CORE - 1:
                nc.scalar.copy(d_sb[:, :512], ps[0][:])
                nc.vector.tensor_copy(d_sb[:, 512:], ps[1][:])
                nc.sync.dma_start(out=d[t * P : (t + 1) * P, :], in_=d_sb[:])
            else:
                # Last tile: split the eviction + store so the pipeline
                # tail after the final matmul is as short as possible.
                nc.scalar.copy(d_sb[:, :512], ps[0][:])
                nc.vector.tensor_copy(d_sb[:, 512:768], ps[1][:, :256])
                nc.scalar.copy(d_sb[:, 768:], ps[1][:, 256:])
                nc.sync.dma_start(
                    out=d[t * P : (t + 1) * P, :768], in_=d_sb[:, :768]
                )
                nc.scalar.dma_start(
                    out=d[t * P : (t + 1) * P, 768:], in_=d_sb[:, 768:]
                )
            # free the chunk dict entries we no longer need
            if j == CHUNK - 1:
                a_ch.pop((c, 0), None)
                a_ch.pop((c, 1), None)

    nc.compile()
    return nc


_NC_CACHE = None


def _prep_inputs(a, b):
    """Host-side shard + transpose + fp8 hi/lo split. Returns in_maps."""
    import ml_dtypes

    f8 = ml_dtypes.float8_e4m3

    a32 = np.ascontiguousarray(np.asarray(a), dtype=np.float32)
    b32 = np.ascontiguousarray(np.asarray(b), dtype=np.float32)
    assert a32.shape == (M_TOTAL, K), a32.shape
    assert b32.shape == (E, K, N), b32.shape

    a_h = a32.astype(f8)
    a_l = (a32 - a_h.astype(np.float32)).astype(f8)
    b_h = b32.astype(f8)
    b_l = (b32 - b_h.astype(np.float32)).astype(f8)

    # Per-expert B in [ki, ko, n] lhs-contraction layout, flattened 2D.
    def prep_b(x):  # x: [K, N] fp8
        return np.ascontiguousarray(
            x.reshape(KK, P, N).transpose(1, 0, 2).reshape(P * KK, N)
        )

    b_h_prep = [prep_b(b_h[e]) for e in range(E)]
    b_l_prep = [prep_b(b_l[e]) for e in range(E)]

    # Per-core A shards (zero-padded), then per-tile transpose to
    # [t, ki, ko, mm] flattened to [(t ki), (ko mm)].
    def prep_a(x):  # x: [ROWS_PER_CORE, K] fp8
        y = x.reshape(TILES_PER_CORE, P, KK, P).transpose(0, 3, 2, 1)
        return np.ascontiguousarray(y).reshape(ROWS_PER_CORE, K)

    in_maps = []
    for c in range(8):
        sh_h = np.zeros((ROWS_PER_CORE, K), dtype=f8)
        sh_l = np.zeros((ROWS_PER_CORE, K), dtype=f8)
        for cc, s, soff, goff, n in _SCHEDULE:
            if cc == c:
                sh_h[soff : soff + n] = a_h[goff : goff + n]
                sh_l[soff : soff + n] = a_l[goff : goff + n]
        m = {"ah": prep_a(sh_h), "al": prep_a(sh_l)}
        for s in range(3):
            e = SLOT_EXPERT[s][c]
            m[f"bh{s}"] = b_h_prep[e]
            m[f"bl{s}"] = b_l_prep[e]
        in_maps.append(m)
    return in_maps


def kernel(a, b):
    global _NC_CACHE
    from concourse.bass_utils import run_bass_kernel_spmd

    if _NC_CACHE is None:
        _NC_CACHE = _build_bass()
    nc = _NC_CACHE

    in_maps = _prep_inputs(a, b)
    res = run_bass_kernel_spmd(nc, in_maps, core_ids=list(range(8)))

    out = np.empty((M_TOTAL, N), dtype=np.float32)
    for c, s, soff, goff, n in _SCHEDULE:
        out[goff : goff + n] = res.results[c]["d"][soff : soff + n].astype(
            np.float32
        )
    return out


# revision 10
# speedup vs baseline: 1.0185x; 1.0185x over previous
"""Grouped GEMM (MoE expert matmul) on 8 TRN2 NeuronCores.

Problem: a [66048, 1024] f32 tokens, b [8, 1024, 1024] f32 expert weights,
static uneven per-expert token counts. d[m] = a[m] @ b[expert(m)].

Strategy (expert-parallel via M-sharding, zero collectives):
- Token rows are assigned host-side to 8 cores x 3 "slots" of (6, 22, 37)
  m-tiles (128 rows each) = 65 tiles/core. Every slot is single-expert;
  each core receives the 3 expert matrices its slots need. The
  (core,slot)->expert binding is pure DATA, so one SPMD program serves
  all cores. Only 4 of 520 tiles are zero-padding.
- A is pre-transposed host-side into per-tile lhsT layout [ki, ko, mm]
  (so the PE does no transposes at all) and split into fp8-e4m3 hi+lo
  (a ~= a_h + a_l); B likewise. The product is computed as
      d ~= a_h@b_h + a_l@b_h + a_h@b_l
  with all three terms as fp8 DoubleRow matmuls (2 k-tiles per
  instruction) accumulating into the same PSUM bank. This is 0.75x the
  PE cost of a bf16 kernel at bf16-level accuracy (~2e-3 rel err).
- Per m-tile: 24 DoubleRow matmuls (2 psum halves x 4 k-pairs x 3
  terms), PSUM evicted to SBUF as bf16 by ScalarE/DVE (one half each),
  stored by HWDGE DMA. A tiles stream in chunks of 5 via SWDGE with
  3-chunk prefetch; d is upcast to f32 host-side.
"""

import numpy as np

GROUP_SIZES = [12288, 10240, 9216, 8192, 7168, 7168, 6144, 5632]
OFFSETS = np.concatenate([[0], np.cumsum(GROUP_SIZES)]).astype(np.int64)
M_TOTAL = int(OFFSETS[-1])  # 66048
K = 1024
N = 1024
E = 8
P = 128
KK = K // P  # 8 k-tiles
NH = 2  # two 512-wide psum halves

# Per-core uniform slot structure, in m-tiles of 128 rows.
SLOT_TILES = (6, 22, 37)  # sum = 65 tiles = 8320 rows per core
TILES_PER_CORE = sum(SLOT_TILES)
ROWS_PER_CORE = TILES_PER_CORE * P
SLOT_ROW_OFF = (0, SLOT_TILES[0] * P, (SLOT_TILES[0] + SLOT_TILES[1]) * P)

CHUNK = 5  # m-tiles per A-load DMA; 13 chunks cover 65 tiles
NCHUNKS = TILES_PER_CORE // CHUNK
PREFETCH = 3  # chunks issued ahead of consumption
WARMUP = 110  # dummy PE matmuls burning the p-state ramp during startup DMA

# expert id for (slot, core): found by exact-cover search; 4 pad tiles total.
SLOT_EXPERT = (
    (1, 3, 4, 4, 5, 5, 6, 6),  # slot 0: 6 tiles each
    (0, 3, 4, 4, 5, 5, 7, 7),  # slot 1: 22 tiles each
    (0, 0, 1, 1, 2, 2, 3, 6),  # slot 2: 37 tiles each
)


def _build_schedule():
    """Returns list of (core, slot, slot_row_start, global_row_start, nrows)."""
    cursor = [int(OFFSETS[e]) for e in range(E)]
    recs = []
    # Deterministic fill order: slot index, then core.
    for s in range(3):
        for c in range(8):
            e = SLOT_EXPERT[s][c]
            cap = SLOT_TILES[s] * P
            take = min(cap, int(OFFSETS[e + 1]) - cursor[e])
            if take > 0:
                recs.append((c, s, SLOT_ROW_OFF[s], cursor[e], take))
                cursor[e] += take
    for e in range(E):
        assert cursor[e] == int(OFFSETS[e + 1]), (e, cursor[e])
    return recs


_SCHEDULE = _build_schedule()


def _build_bass():
    import concourse.bass as bass  # noqa: F401
    import concourse.mybir as mybir
    import concourse.tile as tile
    from concourse import bacc

    f32 = mybir.dt.float32
    bf16 = mybir.dt.bfloat16
    f8 = mybir.dt.float8e4

    nc = bacc.Bacc(
        "TRN2", target_bir_lowering=False, debug=False, enable_asserts=False
    )

    # A in pre-transposed lhsT layout: row (t*128 + ki) holds the 1024
    # values [ko, mm] of tile t; hi and lo fp8 planes.
    ah = nc.dram_tensor("ah", [ROWS_PER_CORE, K], f8, kind="ExternalInput").ap()
    al = nc.dram_tensor("al", [ROWS_PER_CORE, K], f8, kind="ExternalInput").ap()
    # B per slot: row (ki*8 + ko) holds the 1024 n-values; hi and lo.
    bhs = [
        nc.dram_tensor(f"bh{s}", [P * KK, N], f8, kind="ExternalInput").ap()
        for s in range(3)
    ]
    bls = [
        nc.dram_tensor(f"bl{s}", [P * KK, N], f8, kind="ExternalInput").ap()
        for s in range(3)
    ]
    d = nc.dram_tensor("d", [ROWS_PER_CORE, N], bf16, kind="ExternalOutput").ap()

    # which slot (-> b input) each m-tile uses (static, uniform across cores)
    tile_slot = []
    for s in range(3):
        tile_slot += [s] * SLOT_TILES[s]

    from contextlib import ExitStack

    with tile.TileContext(nc) as tc, ExitStack() as ctx:
        bpool = ctx.enter_context(tc.tile_pool(name="bpool", bufs=1))
        ahpool = ctx.enter_context(tc.tile_pool(name="ahpool", bufs=4))
        alpool = ctx.enter_context(tc.tile_pool(name="alpool", bufs=4))
        psd = ctx.enter_context(tc.tile_pool(name="psd", bufs=7, space="PSUM"))
        wps = ctx.enter_context(tc.tile_pool(name="wps", bufs=1, space="PSUM"))
        # Deep store staging: early DMA-engine time is monopolized by the
        # B/A loads, so d-stores queue up; 24 bufs (48KB) of slack keep the
        # eviction copies (and thus PSUM recycling) from backpressuring PE.
        dpool = ctx.enter_context(tc.tile_pool(name="dpool", bufs=24))

        # Warmup: the PE p-state ramps to full clock only after 3us of
        # continuous execution. Dummy DoubleRow matmuls on zeroed tiles
        # keep the PE busy (and ramping) while the first B/A DMAs land,
        # so the real matmul stream starts at full speed with no idle gap.
        warm = ctx.enter_context(tc.tile_pool(name="warm", bufs=1))
        wa = warm.tile([P, 2, P], f8, name="wa")
        wb = warm.tile([P, 2, 256], f8, name="wb")
        nc.vector.memset(wa[:], 0.0)
        nc.vector.memset(wb[:], 0.0)
        wp = wps.tile([P, 256], f32, name="wp")
        for _ in range(WARMUP):
            nc.tensor.matmul(
                wp[:],
                wa[:],
                wb[:],
                start=True,
                stop=True,
                perf_mode=mybir.MatmulPerfMode.DoubleRow,
            )

        b_sb = {}  # (slot, lvl) -> [128, KK, N] fp8 tile

        def load_b(s, lvl):
            src = (bhs if lvl == 0 else bls)[s]
            bt = bpool.tile([P, KK, N], f8, name=f"b{lvl}_{s}")
            nc.gpsimd.dma_start(
                out=bt[:], in_=src.rearrange("(ki ko) n -> ki ko n", ko=KK)
            )
            b_sb[(s, lvl)] = bt

        a_ch = {}  # (chunk, lvl) -> [128, CHUNK, KK, 128] fp8 tile

        def load_chunk(c):
            for lvl, (pool, src) in enumerate(((ahpool, ah), (alpool, al))):
                at = pool.tile([P, CHUNK, KK, P], f8, name=f"a{lvl}")
                nc.gpsimd.dma_start(
                    out=at[:],
                    in_=src[c * CHUNK * P : (c + 1) * CHUNK * P, :].rearrange(
                        "(c ki) (ko mm) -> ki c ko mm", ki=P, ko=KK
                    ),
                )
                a_ch[(c, lvl)] = at

        # Startup: b for slot 0 first, then the first A chunks. The other
        # b loads are deferred into the loop (slot 1 is needed at tile 6,
        # slot 2 at tile 28) so the early DMA engine time is free for the
        # first d-stores instead of being monopolized by weight loads.
        load_b(0, 0)
        load_chunk(0)
        load_b(0, 1)
        load_b(1, 0)
        load_chunk(1)
        load_b(1, 1)
        load_chunk(2)

        for t in range(TILES_PER_CORE):
            c, j = divmod(t, CHUNK)
            if j == 0 and c + PREFETCH < NCHUNKS:
                load_chunk(c + PREFETCH)
            if t == 8:
                load_b(2, 0)
                load_b(2, 1)
            s = tile_slot[t]
            at_h = a_ch[(c, 0)]
            at_l = a_ch[(c, 1)]
            b_h = b_sb[(s, 0)]
            b_l = b_sb[(s, 1)]
            ps = [psd.tile([P, 512], f32, name="ps") for _ in range(NH)]
            for nh in range(NH):
                n0, n1 = nh * 512, (nh + 1) * 512
                idx = 0
                for w_t, r_t in ((at_h, b_h), (at_l, b_h), (at_h, b_l)):
                    for jj in range(KK // 2):
                        nc.tensor.matmul(
                            ps[nh][:],
                            w_t[:, j, 2 * jj : 2 * jj + 2, :],
                            r_t[:, 2 * jj : 2 * jj + 2, n0:n1],
                            start=(idx == 0),
                            stop=(idx == 3 * KK // 2 - 1),
                            perf_mode=mybir.MatmulPerfMode.DoubleRow,
                        )
                        idx += 1
            d_sb = dpool.tile([P, N], bf16, name="d_sb")
            if t < TILES_PER_CORE - 1:
                nc.scalar.copy(d_sb[:, :512], ps[0][:])
                nc.vector.tensor_copy(d_sb[:, 512:], ps[1][:])
                nc.sync.dma_start(out=d[t * P : (t + 1) * P, :], in_=d_sb[:])
            else:
                # Last tile: split the eviction + store so the pipeline
                # tail after the final matmul is as short as possible.
                nc.scalar.copy(d_sb[:, :512], ps[0][:])
                nc.vector.tensor_copy(d_sb[:, 512:768], ps[1][:, :256])
                nc.scalar.copy(d_sb[:, 768:], ps[1][:, 256:])
                nc.sync.dma_start(
                    out=d[t * P : (t + 1) * P, :768], in_=d_sb[:, :768]
                )
                nc.scalar.dma_start(
                    out=d[t * P : (t + 1) * P, 768:], in_=d_sb[:, 768:]
                )
            # free the chunk dict entries we no longer need
            if j == CHUNK - 1:
                a_ch.pop((c, 0), None)
                a_ch.pop((c, 1), None)

    nc.compile()
    return nc


_NC_CACHE = None


def _prep_inputs(a, b):
    """Host-side shard + transpose + fp8 hi/lo split. Returns in_maps."""
    import ml_dtypes

    f8 = ml_dtypes.float8_e4m3

    a32 = np.ascontiguousarray(np.asarray(a), dtype=np.float32)
    b32 = np.ascontiguousarray(np.asarray(b), dtype=np.float32)
    assert a32.shape == (M_TOTAL, K), a32.shape
    assert b32.shape == (E, K, N), b32.shape

    a_h = a32.astype(f8)
    a_l = (a32 - a_h.astype(np.float32)).astype(f8)
    b_h = b32.astype(f8)
    b_l = (b32 - b_h.astype(np.float32)).astype(f8)

    # Per-expert B in [ki, ko, n] lhs-contraction layout, flattened 2D.
    def prep_b(x):  # x: [K, N] fp8
        return np.ascontiguousarray(
            x.reshape(KK, P, N).transpose(1, 0, 2).reshape(P * KK, N)
        )

    b_h_prep = [prep_b(b_h[e]) for e in range(E)]
    b_l_prep = [prep_b(b_l[e]) for e in range(E)]

    # Per-core A shards (zero-padded), then per-tile transpose to
    # [t, ki, ko, mm] flattened to [(t ki), (ko mm)].
    def prep_a(x):  # x: [ROWS_PER_CORE, K] fp8
        y = x.reshape(TILES_PER_CORE, P, KK, P).transpose(0, 3, 2, 1)
        return np.ascontiguousarray(y).reshape(ROWS_PER_CORE, K)

    in_maps = []
    for c in range(8):
        sh_h = np.zeros((ROWS_PER_CORE, K), dtype=f8)
        sh_l = np.zeros((ROWS_PER_CORE, K), dtype=f8)
        for cc, s, soff, goff, n in _SCHEDULE:
            if cc == c:
                sh_h[soff : soff + n] = a_h[goff : goff + n]
                sh_l[soff : soff + n] = a_l[goff : goff + n]
        m = {"ah": prep_a(sh_h), "al": prep_a(sh_l)}
        for s in range(3):
            e = SLOT_EXPERT[s][c]
            m[f"bh{s}"] = b_h_prep[e]
            m[f"bl{s}"] = b_l_prep[e]
        in_maps.append(m)
    return in_maps


def kernel(a, b):
    global _NC_CACHE
    from concourse.bass_utils import run_bass_kernel_spmd

    if _NC_CACHE is None:
        _NC_CACHE = _build_bass()
    nc = _NC_CACHE

    in_maps = _prep_inputs(a, b)
    res = run_bass_kernel_spmd(nc, in_maps, core_ids=list(range(8)))

    out = np.empty((M_TOTAL, N), dtype=np.float32)
    for c, s, soff, goff, n in _SCHEDULE:
        out[goff : goff + n] = res.results[c]["d"][soff : soff + n].astype(
            np.float32
        )
    return out


# revision 12
# speedup vs baseline: 1.0209x; 1.0023x over previous
"""Grouped GEMM (MoE expert matmul) on 8 TRN2 NeuronCores.

Problem: a [66048, 1024] f32 tokens, b [8, 1024, 1024] f32 expert weights,
static uneven per-expert token counts. d[m] = a[m] @ b[expert(m)].

Strategy (expert-parallel via M-sharding, zero collectives):
- Token rows are assigned host-side to 8 cores x 3 "slots" of (6, 22, 37)
  m-tiles (128 rows each) = 65 tiles/core. Every slot is single-expert;
  each core receives the 3 expert matrices its slots need. The
  (core,slot)->expert binding is pure DATA, so one SPMD program serves
  all cores. Only 4 of 520 tiles are zero-padding.
- A is pre-transposed host-side into per-tile lhsT layout [ki, ko, mm]
  (so the PE does no transposes at all) and split into fp8-e4m3 hi+lo
  (a ~= a_h + a_l); B likewise. The product is computed as
      d ~= a_h@b_h + a_l@b_h + a_h@b_l
  with all three terms as fp8 DoubleRow matmuls (2 k-tiles per
  instruction) accumulating into the same PSUM bank. This is 0.75x the
  PE cost of a bf16 kernel at bf16-level accuracy (~2e-3 rel err).
- Per m-tile: 24 DoubleRow matmuls (2 psum halves x 4 k-pairs x 3
  terms), PSUM evicted to SBUF as bf16 by ScalarE/DVE (one half each),
  stored by HWDGE DMA. A tiles stream in chunks of 5 via SWDGE with
  3-chunk prefetch; d is upcast to f32 host-side.
"""

import numpy as np

GROUP_SIZES = [12288, 10240, 9216, 8192, 7168, 7168, 6144, 5632]
OFFSETS = np.concatenate([[0], np.cumsum(GROUP_SIZES)]).astype(np.int64)
M_TOTAL = int(OFFSETS[-1])  # 66048
K = 1024
N = 1024
E = 8
P = 128
KK = K // P  # 8 k-tiles
NH = 2  # two 512-wide psum halves

# Per-core uniform slot structure, in m-tiles of 128 rows.
SLOT_TILES = (6, 22, 37)  # sum = 65 tiles = 8320 rows per core
TILES_PER_CORE = sum(SLOT_TILES)
ROWS_PER_CORE = TILES_PER_CORE * P
SLOT_ROW_OFF = (0, SLOT_TILES[0] * P, (SLOT_TILES[0] + SLOT_TILES[1]) * P)

CHUNK = 5  # m-tiles per A-load DMA; 13 chunks cover 65 tiles
NCHUNKS = TILES_PER_CORE // CHUNK
PREFETCH = 3  # chunks issued ahead of consumption
WARMUP = 90  # dummy PE matmuls burning the p-state ramp during startup DMA

# expert id for (slot, core): found by exact-cover search; 4 pad tiles total.
SLOT_EXPERT = (
    (1, 3, 4, 4, 5, 5, 6, 6),  # slot 0: 6 tiles each
    (0, 3, 4, 4, 5, 5, 7, 7),  # slot 1: 22 tiles each
    (0, 0, 1, 1, 2, 2, 3, 6),  # slot 2: 37 tiles each
)


def _build_schedule():
    """Returns list of (core, slot, slot_row_start, global_row_start, nrows)."""
    cursor = [int(OFFSETS[e]) for e in range(E)]
    recs = []
    # Deterministic fill order: slot index, then core.
    for s in range(3):
        for c in range(8):
            e = SLOT_EXPERT[s][c]
            cap = SLOT_TILES[s] * P
            take = min(cap, int(OFFSETS[e + 1]) - cursor[e])
            if take > 0:
                recs.append((c, s, SLOT_ROW_OFF[s], cursor[e], take))
                cursor[e] += take
    for e in range(E):
        assert cursor[e] == int(OFFSETS[e + 1]), (e, cursor[e])
    return recs


_SCHEDULE = _build_schedule()


def _build_bass():
    import concourse.bass as bass  # noqa: F401
    import concourse.mybir as mybir
    import concourse.tile as tile
    from concourse import bacc

    f32 = mybir.dt.float32
    bf16 = mybir.dt.bfloat16
    f8 = mybir.dt.float8e4

    nc = bacc.Bacc(
        "TRN2", target_bir_lowering=False, debug=False, enable_asserts=False
    )

    # A in pre-transposed lhsT layout: row (t*128 + ki) holds the 1024
    # values [ko, mm] of tile t; hi and lo fp8 planes.
    ah = nc.dram_tensor("ah", [ROWS_PER_CORE, K], f8, kind="ExternalInput").ap()
    al = nc.dram_tensor("al", [ROWS_PER_CORE, K], f8, kind="ExternalInput").ap()
    # B per slot: row (ki*8 + ko) holds the 1024 n-values; hi and lo.
    bhs = [
        nc.dram_tensor(f"bh{s}", [P * KK, N], f8, kind="ExternalInput").ap()
        for s in range(3)
    ]
    bls = [
        nc.dram_tensor(f"bl{s}", [P * KK, N], f8, kind="ExternalInput").ap()
        for s in range(3)
    ]
    d = nc.dram_tensor("d", [ROWS_PER_CORE, N], bf16, kind="ExternalOutput").ap()

    # which slot (-> b input) each m-tile uses (static, uniform across cores)
    tile_slot = []
    for s in range(3):
        tile_slot += [s] * SLOT_TILES[s]

    from contextlib import ExitStack

    with tile.TileContext(nc) as tc, ExitStack() as ctx:
        bpool = ctx.enter_context(tc.tile_pool(name="bpool", bufs=1))
        ahpool = ctx.enter_context(tc.tile_pool(name="ahpool", bufs=4))
        alpool = ctx.enter_context(tc.tile_pool(name="alpool", bufs=4))
        psd = ctx.enter_context(tc.tile_pool(name="psd", bufs=7, space="PSUM"))
        wps = ctx.enter_context(tc.tile_pool(name="wps", bufs=1, space="PSUM"))
        # Deep store staging: early DMA-engine time is monopolized by the
        # B/A loads, so d-stores queue up; 24 bufs (48KB) of slack keep the
        # eviction copies (and thus PSUM recycling) from backpressuring PE.
        dpool = ctx.enter_context(tc.tile_pool(name="dpool", bufs=24))

        # Warmup: the PE p-state ramps to full clock only after 3us of
        # continuous execution. Dummy DoubleRow matmuls on zeroed tiles
        # keep the PE busy (and ramping) while the first B/A DMAs land,
        # so the real matmul stream starts at full speed with no idle gap.
        warm = ctx.enter_context(tc.tile_pool(name="warm", bufs=1))
        wa = warm.tile([P, 2, P], f8, name="wa")
        wb = warm.tile([P, 2, 256], f8, name="wb")
        nc.vector.memset(wa[:], 0.0)
        nc.vector.memset(wb[:], 0.0)
        wp = wps.tile([P, 256], f32, name="wp")
        for _ in range(WARMUP):
            nc.tensor.matmul(
                wp[:],
                wa[:],
                wb[:],
                start=True,
                stop=True,
                perf_mode=mybir.MatmulPerfMode.DoubleRow,
            )

        b_sb = {}  # (slot, lvl) -> [128, KK, N] fp8 tile

        def load_b(s, lvl):
            src = (bhs if lvl == 0 else bls)[s]
            bt = bpool.tile([P, KK, N], f8, name=f"b{lvl}_{s}")
            nc.gpsimd.dma_start(
                out=bt[:], in_=src.rearrange("(ki ko) n -> ki ko n", ko=KK)
            )
            b_sb[(s, lvl)] = bt

        a_ch = {}  # (chunk, lvl) -> [128, CHUNK, KK, 128] fp8 tile

        def load_chunk(c):
            for lvl, (pool, src) in enumerate(((ahpool, ah), (alpool, al))):
                at = pool.tile([P, CHUNK, KK, P], f8, name=f"a{lvl}")
                nc.gpsimd.dma_start(
                    out=at[:],
                    in_=src[c * CHUNK * P : (c + 1) * CHUNK * P, :].rearrange(
                        "(c ki) (ko mm) -> ki c ko mm", ki=P, ko=KK
                    ),
                )
                a_ch[(c, lvl)] = at

        # Startup: b for slot 0 first, then the first A chunks. The other
        # b loads are deferred into the loop (slot 1 is needed at tile 6,
        # slot 2 at tile 28) so the early DMA engine time is free for the
        # first d-stores instead of being monopolized by weight loads.
        load_b(0, 0)
        load_chunk(0)
        load_b(0, 1)
        load_b(1, 0)
        load_chunk(1)
        load_b(1, 1)
        load_chunk(2)

        for t in range(TILES_PER_CORE):
            c, j = divmod(t, CHUNK)
            if j == 0 and c + PREFETCH < NCHUNKS:
                load_chunk(c + PREFETCH)
            if t == 8:
                load_b(2, 0)
                load_b(2, 1)
            s = tile_slot[t]
            at_h = a_ch[(c, 0)]
            at_l = a_ch[(c, 1)]
            b_h = b_sb[(s, 0)]
            b_l = b_sb[(s, 1)]
            ps = [psd.tile([P, 512], f32, name="ps") for _ in range(NH)]
            for nh in range(NH):
                n0, n1 = nh * 512, (nh + 1) * 512
                idx = 0
                for w_t, r_t in ((at_h, b_h), (at_l, b_h), (at_h, b_l)):
                    for jj in range(KK // 2):
                        nc.tensor.matmul(
                            ps[nh][:],
                            w_t[:, j, 2 * jj : 2 * jj + 2, :],
                            r_t[:, 2 * jj : 2 * jj + 2, n0:n1],
                            start=(idx == 0),
                            stop=(idx == 3 * KK // 2 - 1),
                            perf_mode=mybir.MatmulPerfMode.DoubleRow,
                        )
                        idx += 1
            d_sb = dpool.tile([P, N], bf16, name="d_sb")
            if t < TILES_PER_CORE - 1:
                nc.scalar.copy(d_sb[:, :512], ps[0][:])
                nc.vector.tensor_copy(d_sb[:, 512:], ps[1][:])
                nc.sync.dma_start(out=d[t * P : (t + 1) * P, :], in_=d_sb[:])
            else:
                # Last tile: split the nh1 eviction across ScalarE+DVE and
                # store via SWDGE (gpsimd) — the descriptor prep runs early
                # on the idle Pool engine and the trigger path skips the
                # HWDGE + DGE-delay fixed latencies, shortening the
                # pipeline tail after the final matmul.
                nc.scalar.copy(d_sb[:, :512], ps[0][:])
                nc.vector.tensor_copy(d_sb[:, 512:768], ps[1][:, :256])
                nc.scalar.copy(d_sb[:, 768:], ps[1][:, 256:])
                nc.gpsimd.dma_start(out=d[t * P : (t + 1) * P, :], in_=d_sb[:])
            # free the chunk dict entries we no longer need
            if j == CHUNK - 1:
                a_ch.pop((c, 0), None)
                a_ch.pop((c, 1), None)

    nc.compile()
    return nc


_NC_CACHE = None


def _prep_inputs(a, b):
    """Host-side shard + transpose + fp8 hi/lo split. Returns in_maps."""
    import ml_dtypes

    f8 = ml_dtypes.float8_e4m3

    a32 = np.ascontiguousarray(np.asarray(a), dtype=np.float32)
    b32 = np.ascontiguousarray(np.asarray(b), dtype=np.float32)
    assert a32.shape == (M_TOTAL, K), a32.shape
    assert b32.shape == (E, K, N), b32.shape

    a_h = a32.astype(f8)
    a_l = (a32 - a_h.astype(np.float32)).astype(f8)
    b_h = b32.astype(f8)
    b_l = (b32 - b_h.astype(np.float32)).astype(f8)

    # Per-expert B in [ki, ko, n] lhs-contraction layout, flattened 2D.
    def prep_b(x):  # x: [K, N] fp8
        return np.ascontiguousarray(
            x.reshape(KK, P, N).transpose(1, 0, 2).reshape(P * KK, N)
        )

    b_h_prep = [prep_b(b_h[e]) for e in range(E)]
    b_l_prep = [prep_b(b_l[e]) for e in range(E)]

    # Per-core A shards (zero-padded), then per-tile transpose to
    # [t, ki, ko, mm] flattened to [(t ki), (ko mm)].
    def prep_a(x):  # x: [ROWS_PER_CORE, K] fp8
        y = x.reshape(TILES_PER_CORE, P, KK, P).transpose(0, 3, 2, 1)
        return np.ascontiguousarray(y).reshape(ROWS_PER_CORE, K)

    in_maps = []
    for c in range(8):
        sh_h = np.zeros((ROWS_PER_CORE, K), dtype=f8)
        sh_l = np.zeros((ROWS_PER_CORE, K), dtype=f8)
        for cc, s, soff, goff, n in _SCHEDULE:
            if cc == c:
                sh_h[soff : soff + n] = a_h[goff : goff + n]
                sh_l[soff : soff + n] = a_l[goff : goff + n]
        m = {"ah": prep_a(sh_h), "al": prep_a(sh_l)}
        for s in range(3):
            e = SLOT_EXPERT[s][c]
            m[f"bh{s}"] = b_h_prep[e]
            m[f"bl{s}"] = b_l_prep[e]
        in_maps.append(m)
    return in_maps


def kernel(a, b):
    global _NC_CACHE
    from concourse.bass_utils import run_bass_kernel_spmd

    if _NC_CACHE is None:
        _NC_CACHE = _build_bass()
    nc = _NC_CACHE

    in_maps = _prep_inputs(a, b)
    res = run_bass_kernel_spmd(nc, in_maps, core_ids=list(range(8)))

    out = np.empty((M_TOTAL, N), dtype=np.float32)
    for c, s, soff, goff, n in _SCHEDULE:
        out[goff : goff + n] = res.results[c]["d"][soff : soff + n].astype(
            np.float32
        )
    return out


# revision 13
# speedup vs baseline: 1.0259x; 1.0050x over previous
"""Grouped GEMM (MoE expert matmul) on 8 TRN2 NeuronCores.

Problem: a [66048, 1024] f32 tokens, b [8, 1024, 1024] f32 expert weights,
static uneven per-expert token counts. d[m] = a[m] @ b[expert(m)].

Strategy (expert-parallel via M-sharding, zero collectives):
- Token rows are assigned host-side to 8 cores x 3 "slots" of (6, 22, 37)
  m-tiles (128 rows each) = 65 tiles/core. Every slot is single-expert;
  each core receives the 3 expert matrices its slots need. The
  (core,slot)->expert binding is pure DATA, so one SPMD program serves
  all cores. Only 4 of 520 tiles are zero-padding.
- A is pre-transposed host-side into per-tile lhsT layout [ki, ko, mm]
  (so the PE does no transposes at all) and split into fp8-e4m3 hi+lo
  (a ~= a_h + a_l); B likewise. The product is computed as
      d ~= a_h@b_h + a_l@b_h + a_h@b_l
  with all three terms as fp8 DoubleRow matmuls (2 k-tiles per
  instruction) accumulating into the same PSUM bank. This is 0.75x the
  PE cost of a bf16 kernel at bf16-level accuracy (~2e-3 rel err).
- Per m-tile: 24 DoubleRow matmuls (2 psum halves x 4 k-pairs x 3
  terms), PSUM evicted to SBUF as bf16 by ScalarE/DVE (one half each),
  stored by HWDGE DMA. A tiles stream in chunks of 5 via SWDGE with
  3-chunk prefetch; d is upcast to f32 host-side.
"""

import numpy as np

GROUP_SIZES = [12288, 10240, 9216, 8192, 7168, 7168, 6144, 5632]
OFFSETS = np.concatenate([[0], np.cumsum(GROUP_SIZES)]).astype(np.int64)
M_TOTAL = int(OFFSETS[-1])  # 66048
K = 1024
N = 1024
E = 8
P = 128
KK = K // P  # 8 k-tiles
NH = 2  # two 512-wide psum halves

# Per-core uniform slot structure, in m-tiles of 128 rows.
SLOT_TILES = (6, 22, 37)  # sum = 65 tiles = 8320 rows per core
TILES_PER_CORE = sum(SLOT_TILES)
ROWS_PER_CORE = TILES_PER_CORE * P
SLOT_ROW_OFF = (0, SLOT_TILES[0] * P, (SLOT_TILES[0] + SLOT_TILES[1]) * P)

CHUNK = 5  # m-tiles per A-load DMA; 13 chunks cover 65 tiles
NCHUNKS = TILES_PER_CORE // CHUNK
PREFETCH = 3  # chunks issued ahead of consumption
WARMUP = 90  # dummy PE matmuls burning the p-state ramp during startup DMA

# expert id for (slot, core): found by exact-cover search; 4 pad tiles total.
SLOT_EXPERT = (
    (1, 3, 4, 4, 5, 5, 6, 6),  # slot 0: 6 tiles each
    (0, 3, 4, 4, 5, 5, 7, 7),  # slot 1: 22 tiles each
    (0, 0, 1, 1, 2, 2, 3, 6),  # slot 2: 37 tiles each
)


def _build_schedule():
    """Returns list of (core, slot, slot_row_start, global_row_start, nrows)."""
    cursor = [int(OFFSETS[e]) for e in range(E)]
    recs = []
    # Deterministic fill order: slot index, then core.
    for s in range(3):
        for c in range(8):
            e = SLOT_EXPERT[s][c]
            cap = SLOT_TILES[s] * P
            take = min(cap, int(OFFSETS[e + 1]) - cursor[e])
            if take > 0:
                recs.append((c, s, SLOT_ROW_OFF[s], cursor[e], take))
                cursor[e] += take
    for e in range(E):
        assert cursor[e] == int(OFFSETS[e + 1]), (e, cursor[e])
    return recs


_SCHEDULE = _build_schedule()


def _build_bass():
    import concourse.bass as bass  # noqa: F401
    import concourse.mybir as mybir
    import concourse.tile as tile
    from concourse import bacc

    f32 = mybir.dt.float32
    bf16 = mybir.dt.bfloat16
    f8 = mybir.dt.float8e4

    nc = bacc.Bacc(
        "TRN2", target_bir_lowering=False, debug=False, enable_asserts=False
    )

    # A in pre-transposed lhsT layout: row (t*128 + ki) holds the 1024
    # values [ko, mm] of tile t; hi and lo fp8 planes.
    ah = nc.dram_tensor("ah", [ROWS_PER_CORE, K], f8, kind="ExternalInput").ap()
    al = nc.dram_tensor("al", [ROWS_PER_CORE, K], f8, kind="ExternalInput").ap()
    # B per slot: row (ki*8 + ko) holds the 1024 n-values; hi and lo.
    bhs = [
        nc.dram_tensor(f"bh{s}", [P * KK, N], f8, kind="ExternalInput").ap()
        for s in range(3)
    ]
    bls = [
        nc.dram_tensor(f"bl{s}", [P * KK, N], f8, kind="ExternalInput").ap()
        for s in range(3)
    ]
    d = nc.dram_tensor("d", [ROWS_PER_CORE, N], bf16, kind="ExternalOutput").ap()

    # which slot (-> b input) each m-tile uses (static, uniform across cores)
    tile_slot = []
    for s in range(3):
        tile_slot += [s] * SLOT_TILES[s]

    from contextlib import ExitStack

    with tile.TileContext(nc) as tc, ExitStack() as ctx:
        bpool = ctx.enter_context(tc.tile_pool(name="bpool", bufs=1))
        ahpool = ctx.enter_context(tc.tile_pool(name="ahpool", bufs=4))
        alpool = ctx.enter_context(tc.tile_pool(name="alpool", bufs=4))
        psd = ctx.enter_context(tc.tile_pool(name="psd", bufs=7, space="PSUM"))
        wps = ctx.enter_context(tc.tile_pool(name="wps", bufs=1, space="PSUM"))
        # Deep store staging: early DMA-engine time is monopolized by the
        # B/A loads, so d-stores queue up; 24 bufs (48KB) of slack keep the
        # eviction copies (and thus PSUM recycling) from backpressuring PE.
        dpool = ctx.enter_context(tc.tile_pool(name="dpool", bufs=24))

        # Warmup: the PE p-state ramps to full clock only after 3us of
        # continuous execution. Dummy DoubleRow matmuls on zeroed tiles
        # keep the PE busy (and ramping) while the first B/A DMAs land,
        # so the real matmul stream starts at full speed with no idle gap.
        warm = ctx.enter_context(tc.tile_pool(name="warm", bufs=1))
        wa = warm.tile([P, 2, P], f8, name="wa")
        wb = warm.tile([P, 2, 256], f8, name="wb")
        nc.vector.memset(wa[:], 0.0)
        nc.vector.memset(wb[:], 0.0)
        wp = wps.tile([P, 256], f32, name="wp")
        for _ in range(WARMUP):
            nc.tensor.matmul(
                wp[:],
                wa[:],
                wb[:],
                start=True,
                stop=True,
                perf_mode=mybir.MatmulPerfMode.DoubleRow,
            )

        b_sb = {}  # (slot, lvl) -> [128, KK, N] fp8 tile

        def load_b(s, lvl):
            src = (bhs if lvl == 0 else bls)[s]
            bt = bpool.tile([P, KK, N], f8, name=f"b{lvl}_{s}")
            nc.gpsimd.dma_start(
                out=bt[:], in_=src.rearrange("(ki ko) n -> ki ko n", ko=KK)
            )
            b_sb[(s, lvl)] = bt

        a_ch = {}  # (chunk, lvl) -> [128, CHUNK, KK, 128] fp8 tile

        def load_chunk(c):
            for lvl, (pool, src) in enumerate(((ahpool, ah), (alpool, al))):
                at = pool.tile([P, CHUNK, KK, P], f8, name=f"a{lvl}")
                nc.gpsimd.dma_start(
                    out=at[:],
                    in_=src[c * CHUNK * P : (c + 1) * CHUNK * P, :].rearrange(
                        "(c ki) (ko mm) -> ki c ko mm", ki=P, ko=KK
                    ),
                )
                a_ch[(c, lvl)] = at

        # Startup: b for slot 0 first, then the first A chunks. The other
        # b loads are deferred into the loop (slot 1 is needed at tile 6,
        # slot 2 at tile 28) so the early DMA engine time is free for the
        # first d-stores instead of being monopolized by weight loads.
        load_b(0, 0)
        load_chunk(0)
        load_b(0, 1)
        load_b(1, 0)
        load_chunk(1)
        load_b(1, 1)
        load_chunk(2)

        for t in range(TILES_PER_CORE):
            c, j = divmod(t, CHUNK)
            if j == 0 and c + PREFETCH < NCHUNKS:
                load_chunk(c + PREFETCH)
            if t == 8:
                load_b(2, 0)
                load_b(2, 1)
            s = tile_slot[t]
            at_h = a_ch[(c, 0)]
            at_l = a_ch[(c, 1)]
            b_h = b_sb[(s, 0)]
            b_l = b_sb[(s, 1)]
            ps = [psd.tile([P, 512], f32, name="ps") for _ in range(NH)]
            for nh in range(NH):
                n0, n1 = nh * 512, (nh + 1) * 512
                idx = 0
                for w_t, r_t in ((at_h, b_h), (at_l, b_h), (at_h, b_l)):
                    for jj in range(KK // 2):
                        nc.tensor.matmul(
                            ps[nh][:],
                            w_t[:, j, 2 * jj : 2 * jj + 2, :],
                            r_t[:, 2 * jj : 2 * jj + 2, n0:n1],
                            start=(idx == 0),
                            stop=(idx == 3 * KK // 2 - 1),
                            perf_mode=mybir.MatmulPerfMode.DoubleRow,
                        )
                        idx += 1
            d_sb = dpool.tile([P, N], bf16, name="d_sb")
            if t < TILES_PER_CORE - 1:
                nc.scalar.copy(d_sb[:, :512], ps[0][:])
                nc.vector.tensor_copy(d_sb[:, 512:], ps[1][:])
                nc.sync.dma_start(out=d[t * P : (t + 1) * P, :], in_=d_sb[:])
            else:
                # Last tile: store the nh0 half as soon as its chain stops
                # (mid-tile, hiding the ~1.9us DMA issue chain behind the
                # nh1 matmuls) and the nh1 half on the scalar queue right
                # after the prompt DVE eviction, so the post-matmul tail is
                # just copy + one store.
                nc.scalar.copy(d_sb[:, :512], ps[0][:])
                nc.sync.dma_start(
                    out=d[t * P : (t + 1) * P, :512], in_=d_sb[:, :512]
                )
                nc.vector.tensor_copy(d_sb[:, 512:], ps[1][:])
                nc.scalar.dma_start(
                    out=d[t * P : (t + 1) * P, 512:], in_=d_sb[:, 512:]
                )
            # free the chunk dict entries we no longer need
            if j == CHUNK - 1:
                a_ch.pop((c, 0), None)
                a_ch.pop((c, 1), None)

    nc.compile()
    return nc


_NC_CACHE = None


def _prep_inputs(a, b):
    """Host-side shard + transpose + fp8 hi/lo split. Returns in_maps."""
    import ml_dtypes

    f8 = ml_dtypes.float8_e4m3

    a32 = np.ascontiguousarray(np.asarray(a), dtype=np.float32)
    b32 = np.ascontiguousarray(np.asarray(b), dtype=np.float32)
    assert a32.shape == (M_TOTAL, K), a32.shape
    assert b32.shape == (E, K, N), b32.shape

    a_h = a32.astype(f8)
    a_l = (a32 - a_h.astype(np.float32)).astype(f8)
    b_h = b32.astype(f8)
    b_l = (b32 - b_h.astype(np.float32)).astype(f8)

    # Per-expert B in [ki, ko, n] lhs-contraction layout, flattened 2D.
    def prep_b(x):  # x: [K, N] fp8
        return np.ascontiguousarray(
            x.reshape(KK, P, N).transpose(1, 0, 2).reshape(P * KK, N)
        )

    b_h_prep = [prep_b(b_h[e]) for e in range(E)]
    b_l_prep = [prep_b(b_l[e]) for e in range(E)]

    # Per-core A shards (zero-padded), then per-tile transpose to
    # [t, ki, ko, mm] flattened to [(t ki), (ko mm)].
    def prep_a(x):  # x: [ROWS_PER_CORE, K] fp8
        y = x.reshape(TILES_PER_CORE, P, KK, P).transpose(0, 3, 2, 1)
        return np.ascontiguousarray(y).reshape(ROWS_PER_CORE, K)

    in_maps = []
    for c in range(8):
        sh_h = np.zeros((ROWS_PER_CORE, K), dtype=f8)
        sh_l = np.zeros((ROWS_PER_CORE, K), dtype=f8)
        for cc, s, soff, goff, n in _SCHEDULE:
            if cc == c:
                sh_h[soff : soff + n] = a_h[goff : goff + n]
                sh_l[soff : soff + n] = a_l[goff : goff + n]
        m = {"ah": prep_a(sh_h), "al": prep_a(sh_l)}
        for s in range(3):
            e = SLOT_EXPERT[s][c]
            m[f"bh{s}"] = b_h_prep[e]
            m[f"bl{s}"] = b_l_prep[e]
        in_maps.append(m)
    return in_maps


def kernel(a, b):
    global _NC_CACHE
    from concourse.bass_utils import run_bass_kernel_spmd

    if _NC_CACHE is None:
        _NC_CACHE = _build_bass()
    nc = _NC_CACHE

    in_maps = _prep_inputs(a, b)
    res = run_bass_kernel_spmd(nc, in_maps, core_ids=list(range(8)))

    out = np.empty((M_TOTAL, N), dtype=np.float32)
    for c, s, soff, goff, n in _SCHEDULE:
        out[goff : goff + n] = res.results[c]["d"][soff : soff + n].astype(
            np.float32
        )
    return out
